# revision 24
# baseline (speedup 1.0000x reference)
"""AttLoRA MoE-routing kernel for 8 Trainium2 NeuronCores.

Reference computation (per problem nn_AttLoRAModule_85839216378078):
    base  = x @ W_org.T                                    [B,S,OUT]
    q     = x.mean(axis=1) @ Wq.T                          [B,K]
    coef  = softmax(q @ lora_keys.T / sqrt(K))             [B,E]
    h     = x @ lora_down[e]                               [B,S,E,R]
    delta = sum_e coef[b,e] * (h[...,e,:] @ lora_up[e])    [B,S,OUT]
    out   = base + delta * SCALE

Sharding: 8 cores = 4 batches x 2 OUT-halves.  Core c handles batch c//2,
output columns [(c%2)*2048, (c%2+1)*2048).  Each core sees the full x[b], so
the router (softmax coefficients) is computed on-device per core with no
collectives.

Device strategy (per core):
  - All matmuls in float32r (full PE rate at N>=512, ~1e-4 rel err).
  - K(=IN)-split into 2 passes of 2048 contraction rows so the x.T slab
    [2048, 2048] f32 (16 MiB) stays SBUF-resident per pass; W / lora_down
    stream through a shared pool; output accumulated in DRAM via a second
    pass with accum_op=add DMA.
  - LoRA path: tT[er,s] = (x @ lora_down).T accumulated across passes into a
    bf16 tile; router coeffs folded into lora_up tiles (bf16); 4 delta
    matmuls appended to each pass-2 PSUM accumulation group.
  - Router: proj = x @ ((Wq.T @ keys.T)/(S*sqrt(K))) accumulated in one PSUM
    bank across both passes, reduced over S, softmax on one partition, then
    broadcast to partitions via rank-1 outer-product matmuls.
"""

import math
import os

import numpy as np

import concourse.bacc as bacc
import concourse.mybir as mybir
import concourse.tile as tile
from concourse.bass_utils import run_bass_kernel_spmd

# Problem shapes (hardcoded per contest contract)
B, S, IN, OUT = 4, 2048, 4096, 4096
E, R, K = 8, 64, 128
ER = E * R            # 512
OH = OUT // 2         # 2048 output cols per core
P = 128
NP = 2                # contraction passes
IOP = IN // NP // P   # 16 io-subtiles per pass
SCALE = 1.0           # (alpha/lora_dim) * multiplier

F32 = mybir.dt.float32
F32R = mybir.dt.float32r
BF16 = mybir.dt.bfloat16

_NC_CACHE = {}


def _build_nc():
    nc = bacc.Bacc("TRN2", target_bir_lowering=False, debug=False)

    # f32r inputs: same 4-byte fp32 payload, but typed float32r end-to-end so
    # the BIR verifier accepts them as FP32r-matmul operands.
    xT = nc.dram_tensor("xT", [IN, S], F32R, kind="ExternalInput")
    wT = nc.dram_tensor("wT", [IN, OH], F32R, kind="ExternalInput")
    ldn = nc.dram_tensor("ldn", [IN, ER], F32R, kind="ExternalInput")
    lup = nc.dram_tensor("lup", [ER, OH], F32, kind="ExternalInput")
    mk = nc.dram_tensor("mk", [IN, E], F32R, kind="ExternalInput")
    cind = nc.dram_tensor("cind", [E, ER], F32, kind="ExternalInput")
    out = nc.dram_tensor("out", [S, OH], F32, kind="ExternalOutput")

    xT_ap, wT_ap, ldn_ap, lup_ap, mk_ap, cind_ap, out_ap = (
        t.ap() for t in (xT, wT, ldn, lup, mk, cind, out)
    )

    trace_sim = os.environ.get("KERNEL_SIM_TRACE", "0") == "1"
    with tile.TileContext(nc, trace_sim=trace_sim) as tc:
        with (
            tc.tile_pool(name="xpool", bufs=1) as xpool,
            tc.tile_pool(name="spool", bufs=3) as spool,
            tc.tile_pool(name="tpool", bufs=1) as tpool,
            tc.tile_pool(name="lpool", bufs=1) as lpool,
            tc.tile_pool(name="opool", bufs=2) as opool,
            tc.tile_pool(name="rpool", bufs=1) as rpool,
            tc.tile_pool(name="ptp", bufs=2, space="PSUM") as ptp,
            tc.tile_pool(name="pop", bufs=4, space="PSUM") as pop,
            tc.tile_pool(name="prp", bufs=1, space="PSUM") as prp,
            tc.tile_pool(name="pccp", bufs=1, space="PSUM") as pccp,
        ):
            # --- persistent tiles ---
            mk_sb = rpool.tile([P, IN // P, E], F32R, name="mk_sb")
            nc.sync.dma_start(mk_sb[:], mk_ap.rearrange("(io pp) e -> pp io e", pp=P))
            cind_sb = rpool.tile([E, ER], F32, name="cind_sb")
            nc.sync.dma_start(cind_sb[:], cind_ap)

            tT = tpool.tile([P, ER // P, S], BF16, name="tT")
            lsc_all = lpool.tile([P, ER // P, OH], BF16, name="lsc_all")
            pr_t = prp.tile([E, 512], F32, name="pr_t")
            ones8 = rpool.tile([E, 1], F32, name="ones8")
            nc.any.memset(ones8[:], 1.0)
            ones_row = rpool.tile([1, P], F32, name="ones_row")
            nc.any.memset(ones_row[:], 1.0)
            coeff_cols = rpool.tile([P, ER // P], F32, name="coeff_cols")

            SC = S // 512  # 4 s-chunks

            for p in range(NP):
                i0 = p * (IN // NP)
                # --- load x in eighths (each [256 i, S]) for DMA-queue
                # parallelism and early phase-T start ---
                xq = []
                for k in range(8):
                    xqk = xpool.tile([P, 2, S], F32R, tag=f"xq{k}", name=f"xq{k}_{p}")
                    nc.sync.dma_start(
                        xqk[:],
                        xT_ap[i0 + k * 256 : i0 + (k + 1) * 256, :].rearrange(
                            "(io pp) s -> pp io s", pp=P
                        ),
                    )
                    xq.append(xqk)

                def xs(io, fslice):
                    return xq[io // 2][:, io % 2, fslice]

                # --- phase T: tT += (ldn_pass.T @ x_pass), chunked by 8 io ---
                for j in range(2):
                    ldc = spool.tile([P, 8, ER], F32R, tag="stream", name=f"ldc_{p}_{j}")
                    nc.sync.dma_start(
                        ldc[:],
                        ldn_ap[i0 + j * 1024 : i0 + (j + 1) * 1024, :].rearrange(
                            "(io pp) e -> pp io e", pp=P
                        ),
                    )
                    for u in range(ER // P):
                        for c in range(SC):
                            ps = ptp.tile([P, 512], F32, tag="pt", name=f"pt_{p}_{j}_{u}_{c}")
                            for jo in range(8):
                                io = j * 8 + jo
                                nc.tensor.matmul(
                                    ps[:],
                                    ldc[:, jo, u * P : (u + 1) * P],
                                    xs(io, slice(c * 512, (c + 1) * 512)),
                                    start=(jo == 0),
                                    stop=(jo == 7),
                                )
                            dst = tT[:, u, c * 512 : (c + 1) * 512]
                            if p == 0 and j == 0:
                                nc.vector.tensor_copy(dst, ps[:])
                            else:
                                nc.vector.tensor_tensor(
                                    dst, dst, ps[:], mybir.AluOpType.add
                                )

                # --- router projection: pr_t[e, j] += sum_i x[s,i] mk[i,e] ---
                for c in range(SC):
                    for io in range(IOP):
                        nc.tensor.matmul(
                            pr_t[:],
                            mk_sb[:, p * IOP + io, :],
                            xs(io, slice(c * 512, (c + 1) * 512)),
                            start=(p == 0 and c == 0 and io == 0),
                            stop=(p == NP - 1 and c == SC - 1 and io == IOP - 1),
                        )

                if p == NP - 1:
                    # --- router finalize (on-device softmax) ---
                    scores = rpool.tile([E, 1], F32, name="scores")
                    nc.vector.reduce_sum(scores[:], pr_t[:], axis=mybir.AxisListType.X)
                    exps = rpool.tile([E, 1], F32, name="exps")
                    nc.scalar.activation(
                        exps[:], scores[:], mybir.ActivationFunctionType.Exp
                    )
                    # sum(exp) via PE partition reduction -> [1, 1]
                    psum_s = pccp.tile([1, 1], F32, tag="pcc", name="psum_s")
                    nc.tensor.matmul(psum_s[:], exps[:], ones8[:], start=True, stop=True)
                    rinv = rpool.tile([1, 1], F32, name="rinv")
                    nc.vector.reciprocal(rinv[:], psum_s[:])
                    # broadcast 1/sum to all 128 partitions via rank-1 outer product
                    rb_p = pccp.tile([P, 1], F32, tag="pcc", name="rb_p")
                    nc.tensor.matmul(rb_p[:], ones_row[:], rinv[:], start=True, stop=True)
                    rb = rpool.tile([P, 1], F32, name="rb")
                    nc.vector.tensor_copy(rb[:], rb_p[:])
                    # partition placement: cc_un[pp, u] = exp(score[(u*128+pp)//64])
                    cc_un = rpool.tile([P, ER // P], F32, name="cc_un")
                    for u in range(ER // P):
                        pcc = pccp.tile([P, 1], F32, tag="pcc", name=f"pcc_{u}")
                        nc.tensor.matmul(
                            pcc[:],
                            cind_sb[:, u * P : (u + 1) * P],
                            exps[:],
                            start=True,
                            stop=True,
                        )
                        nc.vector.tensor_copy(cc_un[:, u : u + 1], pcc[:])
                    # coeff_cols = cc_un / sum(exp) * SCALE  (SCALE == 1.0)
                    nc.vector.tensor_scalar_mul(coeff_cols[:], cc_un[:], rb[:])

                    # pre-scale all of lup into bf16 (coeff folded in) so the
                    # o-loop has no per-n lup load/scale serialization
                    for nn_ in range(OH // 512):
                        lraw = spool.tile(
                            [P, ER // P, 512], F32, tag="stream", name=f"lraw_{nn_}"
                        )
                        nc.sync.dma_start(
                            lraw[:],
                            lup_ap[:, nn_ * 512 : (nn_ + 1) * 512].rearrange(
                                "(u pp) o -> pp u o", pp=P
                            ),
                        )
                        nc.vector.tensor_tensor(
                            lsc_all[:, :, nn_ * 512 : (nn_ + 1) * 512],
                            lraw[:],
                            coeff_cols[:, :, None].to_broadcast((P, ER // P, 512)),
                            mybir.AluOpType.mult,
                        )

                # --- main output loop ---
                for n in range(OH // 512):
                    wc = []
                    for j in range(2):
                        wcj = spool.tile(
                            [P, 8, 512], F32R, tag="stream", name=f"wc_{p}_{n}_{j}"
                        )
                        nc.sync.dma_start(
                            wcj[:],
                            wT_ap[
                                i0 + j * 1024 : i0 + (j + 1) * 1024,
                                n * 512 : (n + 1) * 512,
                            ].rearrange("(io pp) o -> pp io o", pp=P),
                        )
                        wc.append(wcj)

                    for m in range(S // P):
                        po_t = pop.tile([P, 512], F32, tag="po", name=f"po_{p}_{n}_{m}")
                        for io in range(IOP):
                            nc.tensor.matmul(
                                po_t[:],
                                xs(io, slice(m * P, (m + 1) * P)),
                                wc[io // 8][:, io % 8, :],
                                start=(io == 0),
                                stop=(io == IOP - 1 and p < NP - 1),
                            )
                        if p == NP - 1:
                            for u in range(ER // P):
                                nc.tensor.matmul(
                                    po_t[:],
                                    tT[:, u, m * P : (m + 1) * P],
                                    lsc_all[:, u, n * 512 : (n + 1) * 512],
                                    start=False,
                                    stop=(u == ER // P - 1),
                                )
                        ost = opool.tile([P, 512], F32, tag="ost", name=f"ost_{p}_{n}_{m}")
                        nc.vector.tensor_copy(ost[:], po_t[:])
                        dst = out_ap[m * P : (m + 1) * P, n * 512 : (n + 1) * 512]
                        if p == 0:
                            nc.sync.dma_start(dst, ost[:])
                        else:
                            nc.gpsimd.dma_start(dst, ost[:], accum_op=mybir.AluOpType.add)

    nc.compile()
    return nc


def kernel(x, W_org, lora_down, lora_up, lora_keys, Wq):
    x = np.ascontiguousarray(np.asarray(x, dtype=np.float32))
    W_org = np.asarray(W_org, dtype=np.float32)
    lora_down = np.asarray(lora_down, dtype=np.float32)
    lora_up = np.asarray(lora_up, dtype=np.float32)
    lora_keys = np.asarray(lora_keys, dtype=np.float32)
    Wq = np.asarray(Wq, dtype=np.float32)

    # Host-side constant folding / layout prep (transposes to K-major)
    wT_full = np.ascontiguousarray(W_org.T)                          # [IN, OUT]
    ldn = np.ascontiguousarray(lora_down.transpose(1, 0, 2).reshape(IN, ER))
    lup_full = np.ascontiguousarray(lora_up.reshape(ER, OUT))
    mk = np.ascontiguousarray(
        (Wq.T @ lora_keys.T) / (S * math.sqrt(K))
    ).astype(np.float32)                                             # [IN, E]
    cind = np.repeat(np.eye(E, dtype=np.float32), R, axis=1)         # [E, ER]
    xT = [np.ascontiguousarray(x[b].T) for b in range(B)]            # [IN, S]
    wT_half = [np.ascontiguousarray(wT_full[:, h * OH : (h + 1) * OH]) for h in range(2)]
    lup_half = [np.ascontiguousarray(lup_full[:, h * OH : (h + 1) * OH]) for h in range(2)]

    if "nc" not in _NC_CACHE:
        _NC_CACHE["nc"] = _build_nc()
    nc = _NC_CACHE["nc"]

    in_maps = []
    for c in range(8):
        b, h = c // 2, c % 2
        in_maps.append(
            {
                "xT": xT[b],
                "wT": wT_half[h],
                "ldn": ldn,
                "lup": lup_half[h],
                "mk": mk,
                "cind": cind,
            }
        )

    res = run_bass_kernel_spmd(nc, in_maps, core_ids=list(range(8)), trace=False)
    _NC_CACHE["last_result"] = res
    _NC_CACHE["last_in_maps"] = in_maps

    outp = np.empty((B, S, OUT), dtype=np.float32)
    for c in range(8):
        b, h = c // 2, c % 2
        outp[b, :, h * OH : (h + 1) * OH] = res.results[c]["out"]
    return outp


def benchmark(iters: int = 8):
    """Time device execution with inputs pre-placed on the 8 cores.

    Mirrors bass2jax.run_bass_via_pjrt's multi-core shard_map path but keeps
    the non-donated inputs resident on device so the timed region is
    dispatch + NEFF execution only.  Returns per-iteration seconds.
    """
    import time

    import jax
    from jax.experimental.shard_map import shard_map
    from jax.sharding import Mesh, NamedSharding, PartitionSpec

    from concourse import bass2jax, mybir as _mybir

    nc = _NC_CACHE["nc"]
    in_maps = _NC_CACHE["last_in_maps"]
    n_cores = len(in_maps)

    bass2jax.install_neuronx_cc_hook()

    partition_name = nc.partition_id_tensor.name if nc.partition_id_tensor else None
    in_names, out_names, out_avals, zero_outs = [], [], [], []
    for alloc in nc.m.functions[0].allocations:
        if not isinstance(alloc, _mybir.MemoryLocationSet):
            continue
        name = alloc.memorylocations[0].name
        if alloc.kind == "ExternalInput":
            if name != partition_name:
                in_names.append(name)
        elif alloc.kind == "ExternalOutput":
            aval = jax.core.ShapedArray(
                tuple(alloc.tensor_shape), _mybir.dt.np(alloc.dtype)
            )
            out_avals.append(aval)
            out_names.append(name)
            zero_outs.append(np.zeros(aval.shape, aval.dtype))
    n_params = len(in_names)
    n_outs = len(out_avals)
    all_in_names = in_names + out_names
    if partition_name is not None:
        all_in_names = all_in_names + [partition_name]

    def _body(*args):
        operands = list(args)
        if partition_name is not None:
            operands.append(bass2jax.partition_id_tensor())
        outs = bass2jax._bass_exec_p.bind(
            *operands,
            out_avals=tuple(out_avals),
            in_names=tuple(all_in_names),
            out_names=tuple(out_names),
            lowering_input_output_aliases=(),
            sim_require_finite=True,
            sim_require_nnan=True,
            nc=nc,
        )
        return tuple(outs)

    _body.__name__ = "_body"

    devices = jax.devices()[:n_cores]
    mesh = Mesh(np.asarray(devices), ("core",))
    spec = PartitionSpec("core")
    sharding = NamedSharding(mesh, spec)
    donate = tuple(range(n_params, n_params + n_outs))
    fn = jax.jit(
        shard_map(
            _body,
            mesh=mesh,
            in_specs=(spec,) * (n_params + n_outs),
            out_specs=(spec,) * n_outs,
            check_rep=False,
        ),
        donate_argnums=donate,
        keep_unused=True,
    )

    concat_in = [
        np.concatenate([np.asarray(in_maps[c][nm]) for c in range(n_cores)], axis=0)
        for nm in in_names
    ]
    concat_zero = [
        np.zeros((n_cores * z.shape[0], *z.shape[1:]), z.dtype) for z in zero_outs
    ]
    dev_in = [jax.device_put(a, sharding) for a in concat_in]
    for a in dev_in:
        a.block_until_ready()

    times = []
    for _ in range(iters + 1):
        dev_zero = [jax.device_put(z, sharding) for z in concat_zero]
        for z in dev_zero:
            z.block_until_ready()
        t0 = time.perf_counter()
        outs = fn(*dev_in, *dev_zero)
        for o in outs:
            o.block_until_ready()
        times.append(time.perf_counter() - t0)
    return times[1:]  # drop warmup/compile call


# revision 43
# speedup vs baseline: 37.6397x; 37.6397x over previous
"""AttLoRA MoE-routing kernel for 8 Trainium2 NeuronCores.

Reference computation (per problem nn_AttLoRAModule_85839216378078):
    base  = x @ W_org.T                                    [B,S,OUT]
    q     = x.mean(axis=1) @ Wq.T                          [B,K]
    coef  = softmax(q @ lora_keys.T / sqrt(K))             [B,E]
    h     = x @ lora_down[e]                               [B,S,E,R]
    delta = sum_e coef[b,e] * (h[...,e,:] @ lora_up[e])    [B,S,OUT]
    out   = base + delta * SCALE

Sharding: 8 cores = 4 batches x 2 OUT-halves.  Core c handles batch c//2,
output columns [(c%2)*2048, (c%2+1)*2048).  Each core sees the full x[b], so
the router (softmax coefficients) is computed on-device per core with no
collectives.

Device strategy (per core):
  - All matmuls in float32r (full PE rate at N>=512, ~1e-4 rel err).
  - K(=IN)-split into 2 passes of 2048 contraction rows so the x.T slab
    [2048, 2048] f32 (16 MiB) stays SBUF-resident per pass; W / lora_down
    stream through a shared pool; output accumulated in DRAM via a second
    pass with accum_op=add DMA.
  - LoRA path: tT[er,s] = (x @ lora_down).T accumulated across passes into a
    bf16 tile; router coeffs folded into lora_up tiles (bf16); 4 delta
    matmuls appended to each pass-2 PSUM accumulation group.
  - Router: proj = x @ ((Wq.T @ keys.T)/(S*sqrt(K))) accumulated in one PSUM
    bank across both passes, reduced over S, softmax on one partition, then
    broadcast to partitions via rank-1 outer-product matmuls.
"""

import math
import os

import numpy as np

import concourse.bacc as bacc
import concourse.mybir as mybir
import concourse.tile as tile
from concourse.bass_utils import run_bass_kernel_spmd

# Problem shapes (hardcoded per contest contract)
B, S, IN, OUT = 4, 2048, 4096, 4096
E, R, K = 8, 64, 128
ER = E * R            # 512
OH = OUT // 2         # 2048 output cols per core
P = 128
NP = 2                # contraction passes
IOP = IN // NP // P   # 16 io-subtiles per pass
SCALE = 1.0           # (alpha/lora_dim) * multiplier

F32 = mybir.dt.float32
F32R = mybir.dt.float32r
BF16 = mybir.dt.bfloat16

_NC_CACHE = {}


def _build_nc():
    nc = bacc.Bacc("TRN2", target_bir_lowering=False, debug=False)

    # f32r inputs: same 4-byte fp32 payload, but typed float32r end-to-end so
    # the BIR verifier accepts them as FP32r-matmul operands.
    xT = nc.dram_tensor("xT", [IN, S], F32R, kind="ExternalInput")
    wT = nc.dram_tensor("wT", [IN, OH], F32R, kind="ExternalInput")
    ldn = nc.dram_tensor("ldn", [IN, ER], F32R, kind="ExternalInput")
    lup = nc.dram_tensor("lup", [ER, OH], F32, kind="ExternalInput")
    mk = nc.dram_tensor("mk", [IN, E], F32R, kind="ExternalInput")
    cind = nc.dram_tensor("cind", [E, ER], F32, kind="ExternalInput")
    out = nc.dram_tensor("out", [S, OH], F32, kind="ExternalOutput")

    xT_ap, wT_ap, ldn_ap, lup_ap, mk_ap, cind_ap, out_ap = (
        t.ap() for t in (xT, wT, ldn, lup, mk, cind, out)
    )

    trace_sim = os.environ.get("KERNEL_SIM_TRACE", "0") == "1"
    with tile.TileContext(nc, trace_sim=trace_sim) as tc:
        with (
            tc.tile_pool(name="xpool", bufs=1) as xpool,
            tc.tile_pool(name="spool", bufs=3) as spool,
            tc.tile_pool(name="tpool", bufs=1) as tpool,
            tc.tile_pool(name="lpool", bufs=2) as lpool,
            tc.tile_pool(name="opool", bufs=2) as opool,
            tc.tile_pool(name="rpool", bufs=1) as rpool,
            tc.tile_pool(name="ptp", bufs=2, space="PSUM") as ptp,
            tc.tile_pool(name="pop", bufs=4, space="PSUM") as pop,
            tc.tile_pool(name="prp", bufs=1, space="PSUM") as prp,
            tc.tile_pool(name="pccp", bufs=1, space="PSUM") as pccp,
        ):
            # --- persistent tiles ---
            mk_sb = rpool.tile([P, IN // P, E], F32R, name="mk_sb")
            nc.sync.dma_start(mk_sb[:], mk_ap.rearrange("(io pp) e -> pp io e", pp=P))
            cind_sb = rpool.tile([E, ER], F32, name="cind_sb")
            nc.sync.dma_start(cind_sb[:], cind_ap)

            tT = tpool.tile([P, ER // P, S], BF16, name="tT")
            pr_t = prp.tile([E, 512], F32, name="pr_t")
            ones8 = rpool.tile([E, 1], F32, name="ones8")
            nc.any.memset(ones8[:], 1.0)
            ones_row = rpool.tile([1, P], F32, name="ones_row")
            nc.any.memset(ones_row[:], 1.0)
            coeff_cols = rpool.tile([P, ER // P], F32, name="coeff_cols")

            SC = S // 512  # 4 s-chunks

            for p in range(NP):
                i0 = p * (IN // NP)
                # --- phase-T weights first: the very first matmul needs ldc0 ---
                def load_ldc(j):
                    ldcj = spool.tile([P, 8, ER], F32R, tag="stream", name=f"ldc_{p}_{j}")
                    nc.sync.dma_start(
                        ldcj[:],
                        ldn_ap[i0 + j * 1024 : i0 + (j + 1) * 1024, :].rearrange(
                            "(io pp) e -> pp io e", pp=P
                        ),
                    )
                    return ldcj

                ldcs = [load_ldc(0)]

                # --- load x in eighths (each [256 i, S]) for DMA-queue
                # parallelism and early phase-T start ---
                xq = []
                for k in range(8):
                    xqk = xpool.tile([P, 2, S], F32R, tag=f"xq{k}", name=f"xq{k}_{p}")
                    # alternate SWDGE/HWDGE so x streams over both DMA paths;
                    # xq0 goes on gpsimd so it loads in parallel with ldc0 (sync)
                    eng = nc.gpsimd if k % 2 == 0 else nc.sync
                    eng.dma_start(
                        xqk[:],
                        xT_ap[i0 + k * 256 : i0 + (k + 1) * 256, :].rearrange(
                            "(io pp) s -> pp io s", pp=P
                        ),
                    )
                    xq.append(xqk)

                def xs(io, fslice):
                    return xq[io // 2][:, io % 2, fslice]

                ldcs.append(load_ldc(1))  # needed only at phase-T midpoint

                # --- phase T: tT += (ldn_pass.T @ x_pass), chunked by 8 io ---
                for j in range(2):
                    ldc = ldcs[j]
                    for u in range(ER // P):
                        for c in range(SC):
                            ps = ptp.tile([P, 512], F32, tag="pt", name=f"pt_{p}_{j}_{u}_{c}")
                            for jo in range(8):
                                io = j * 8 + jo
                                nc.tensor.matmul(
                                    ps[:],
                                    ldc[:, jo, u * P : (u + 1) * P],
                                    xs(io, slice(c * 512, (c + 1) * 512)),
                                    start=(jo == 0),
                                    stop=(jo == 7),
                                )
                            dst = tT[:, u, c * 512 : (c + 1) * 512]
                            if p == 0 and j == 0:
                                nc.vector.tensor_copy(dst, ps[:])
                            else:
                                nc.vector.tensor_tensor(
                                    dst, dst, ps[:], mybir.AluOpType.add
                                )

                # --- router projection: pr_t[e, j] += sum_i x[s,i] mk[i,e] ---
                for c in range(SC):
                    for io in range(IOP):
                        nc.tensor.matmul(
                            pr_t[:],
                            mk_sb[:, p * IOP + io, :],
                            xs(io, slice(c * 512, (c + 1) * 512)),
                            start=(p == 0 and c == 0 and io == 0),
                            stop=(p == NP - 1 and c == SC - 1 and io == IOP - 1),
                        )

                if p == NP - 1:
                    # --- router finalize (on-device softmax) ---
                    scores = rpool.tile([E, 1], F32, name="scores")
                    nc.vector.reduce_sum(scores[:], pr_t[:], axis=mybir.AxisListType.X)
                    exps = rpool.tile([E, 1], F32, name="exps")
                    nc.scalar.activation(
                        exps[:], scores[:], mybir.ActivationFunctionType.Exp
                    )
                    # sum(exp) via PE partition reduction -> [1, 1]
                    psum_s = pccp.tile([1, 1], F32, tag="pcc", name="psum_s")
                    nc.tensor.matmul(psum_s[:], exps[:], ones8[:], start=True, stop=True)
                    rinv = rpool.tile([1, 1], F32, name="rinv")
                    nc.vector.reciprocal(rinv[:], psum_s[:])
                    # broadcast 1/sum to all 128 partitions via rank-1 outer product
                    rb_p = pccp.tile([P, 1], F32, tag="pcc", name="rb_p")
                    nc.tensor.matmul(rb_p[:], ones_row[:], rinv[:], start=True, stop=True)
                    rb = rpool.tile([P, 1], F32, name="rb")
                    nc.vector.tensor_copy(rb[:], rb_p[:])
                    # partition placement: cc_un[pp, u] = exp(score[(u*128+pp)//64])
                    cc_un = rpool.tile([P, ER // P], F32, name="cc_un")
                    for u in range(ER // P):
                        pcc = pccp.tile([P, 1], F32, tag="pcc", name=f"pcc_{u}")
                        nc.tensor.matmul(
                            pcc[:],
                            cind_sb[:, u * P : (u + 1) * P],
                            exps[:],
                            start=True,
                            stop=True,
                        )
                        nc.vector.tensor_copy(cc_un[:, u : u + 1], pcc[:])
                    # coeff_cols = cc_un / sum(exp) * SCALE  (SCALE == 1.0)
                    nc.vector.tensor_scalar_mul(coeff_cols[:], cc_un[:], rb[:])

                lsc_tiles = [None] * (OH // 512)

                def load_lsc(nn_):
                    lraw = spool.tile(
                        [P, ER // P, 512], F32, tag="stream", name=f"lraw_{nn_}"
                    )
                    nc.gpsimd.dma_start(
                        lraw[:],
                        lup_ap[:, nn_ * 512 : (nn_ + 1) * 512].rearrange(
                            "(u pp) o -> pp u o", pp=P
                        ),
                    )
                    t = lpool.tile([P, ER // P, 512], BF16, tag="lsc", name=f"lsc_{nn_}")
                    nc.vector.tensor_tensor(
                        t[:],
                        lraw[:],
                        coeff_cols[:, :, None].to_broadcast((P, ER // P, 512)),
                        mybir.AluOpType.mult,
                    )
                    lsc_tiles[nn_] = t

                if p == NP - 1:
                    load_lsc(0)

                # --- main output loop ---
                for n in range(OH // 512):
                    wc = []
                    for j in range(2):
                        wcj = spool.tile(
                            [P, 8, 512], F32R, tag="stream", name=f"wc_{p}_{n}_{j}"
                        )
                        (nc.sync if j == 0 else nc.gpsimd).dma_start(
                            wcj[:],
                            wT_ap[
                                i0 + j * 1024 : i0 + (j + 1) * 1024,
                                n * 512 : (n + 1) * 512,
                            ].rearrange("(io pp) o -> pp io o", pp=P),
                        )
                        wc.append(wcj)
                    if p == NP - 1:
                        lsc = lsc_tiles[n]
                        if n + 1 < OH // 512:
                            # prefetch next n's scaled lup during this m-loop
                            load_lsc(n + 1)

                    for m in range(S // P):
                        po_t = pop.tile([P, 512], F32, tag="po", name=f"po_{p}_{n}_{m}")
                        for io in range(IOP):
                            nc.tensor.matmul(
                                po_t[:],
                                xs(io, slice(m * P, (m + 1) * P)),
                                wc[io // 8][:, io % 8, :],
                                start=(io == 0),
                                stop=(io == IOP - 1 and p < NP - 1),
                            )
                        if p == NP - 1:
                            for u in range(ER // P):
                                nc.tensor.matmul(
                                    po_t[:],
                                    tT[:, u, m * P : (m + 1) * P],
                                    lsc[:, u, :],
                                    start=False,
                                    stop=(u == ER // P - 1),
                                )
                        ost = opool.tile([P, 512], F32, tag="ost", name=f"ost_{p}_{n}_{m}")
                        nc.vector.tensor_copy(ost[:], po_t[:])
                        dst = out_ap[m * P : (m + 1) * P, n * 512 : (n + 1) * 512]
                        if p == 0:
                            nc.sync.dma_start(dst, ost[:])
                        else:
                            nc.gpsimd.dma_start(dst, ost[:], accum_op=mybir.AluOpType.add)

    nc.compile()
    return nc


def kernel(x, W_org, lora_down, lora_up, lora_keys, Wq):
    x = np.ascontiguousarray(np.asarray(x, dtype=np.float32))
    W_org = np.asarray(W_org, dtype=np.float32)
    lora_down = np.asarray(lora_down, dtype=np.float32)
    lora_up = np.asarray(lora_up, dtype=np.float32)
    lora_keys = np.asarray(lora_keys, dtype=np.float32)
    Wq = np.asarray(Wq, dtype=np.float32)

    # Host-side constant folding / layout prep (transposes to K-major)
    wT_full = np.ascontiguousarray(W_org.T)                          # [IN, OUT]
    ldn = np.ascontiguousarray(lora_down.transpose(1, 0, 2).reshape(IN, ER))
    lup_full = np.ascontiguousarray(lora_up.reshape(ER, OUT))
    mk = np.ascontiguousarray(
        (Wq.T @ lora_keys.T) / (S * math.sqrt(K))
    ).astype(np.float32)                                             # [IN, E]
    cind = np.repeat(np.eye(E, dtype=np.float32), R, axis=1)         # [E, ER]
    xT = [np.ascontiguousarray(x[b].T) for b in range(B)]            # [IN, S]
    wT_half = [np.ascontiguousarray(wT_full[:, h * OH : (h + 1) * OH]) for h in range(2)]
    lup_half = [np.ascontiguousarray(lup_full[:, h * OH : (h + 1) * OH]) for h in range(2)]

    if "nc" not in _NC_CACHE:
        _NC_CACHE["nc"] = _build_nc()
    nc = _NC_CACHE["nc"]

    in_maps = []
    for c in range(8):
        b, h = c // 2, c % 2
        in_maps.append(
            {
                "xT": xT[b],
                "wT": wT_half[h],
                "ldn": ldn,
                "lup": lup_half[h],
                "mk": mk,
                "cind": cind,
            }
        )

    res = run_bass_kernel_spmd(nc, in_maps, core_ids=list(range(8)), trace=False)
    _NC_CACHE["last_result"] = res
    _NC_CACHE["last_in_maps"] = in_maps

    outp = np.empty((B, S, OUT), dtype=np.float32)
    for c in range(8):
        b, h = c // 2, c % 2
        outp[b, :, h * OH : (h + 1) * OH] = res.results[c]["out"]
    return outp


def _build_baseline_nc():
    """Same I/O signature as the real kernel, near-zero device work.

    Used to measure the fixed dispatch/relay overhead of one execution so the
    real kernel's device time can be estimated as (full - baseline)."""
    nc = bacc.Bacc("TRN2", target_bir_lowering=False, debug=False)
    xT = nc.dram_tensor("xT", [IN, S], F32R, kind="ExternalInput")
    wT = nc.dram_tensor("wT", [IN, OH], F32R, kind="ExternalInput")
    ldn = nc.dram_tensor("ldn", [IN, ER], F32R, kind="ExternalInput")
    lup = nc.dram_tensor("lup", [ER, OH], F32, kind="ExternalInput")
    mk = nc.dram_tensor("mk", [IN, E], F32R, kind="ExternalInput")
    cind = nc.dram_tensor("cind", [E, ER], F32, kind="ExternalInput")
    out = nc.dram_tensor("out", [S, OH], F32, kind="ExternalOutput")
    with tile.TileContext(nc) as tc:
        with tc.tile_pool(name="bp", bufs=1) as bp:
            t = bp.tile([P, 512], F32, name="t")
            nc.sync.dma_start(t[:], lup.ap()[:P, :512])
            nc.sync.dma_start(out.ap()[:P, :512], t[:])
            _ = (xT, wT, ldn, mk, cind)
    nc.compile()
    return nc


def benchmark_baseline(iters: int = 8):
    if "bnc" not in _NC_CACHE:
        _NC_CACHE["bnc"] = _build_baseline_nc()
    return benchmark(iters, nc=_NC_CACHE["bnc"])


def benchmark(iters: int = 8, nc=None):
    """Time device execution with inputs pre-placed on the 8 cores.

    Mirrors bass2jax.run_bass_via_pjrt's multi-core shard_map path but keeps
    the non-donated inputs resident on device so the timed region is
    dispatch + NEFF execution only.  Returns per-iteration seconds.
    """
    import time

    import jax
    from jax.experimental.shard_map import shard_map
    from jax.sharding import Mesh, NamedSharding, PartitionSpec

    from concourse import bass2jax, mybir as _mybir

    if nc is None:
        nc = _NC_CACHE["nc"]
    in_maps = _NC_CACHE["last_in_maps"]
    n_cores = len(in_maps)

    bass2jax.install_neuronx_cc_hook()

    partition_name = nc.partition_id_tensor.name if nc.partition_id_tensor else None
    in_names, out_names, out_avals, zero_outs = [], [], [], []
    for alloc in nc.m.functions[0].allocations:
        if not isinstance(alloc, _mybir.MemoryLocationSet):
            continue
        name = alloc.memorylocations[0].name
        if alloc.kind == "ExternalInput":
            if name != partition_name:
                in_names.append(name)
        elif alloc.kind == "ExternalOutput":
            aval = jax.core.ShapedArray(
                tuple(alloc.tensor_shape), _mybir.dt.np(alloc.dtype)
            )
            out_avals.append(aval)
            out_names.append(name)
            zero_outs.append(np.zeros(aval.shape, aval.dtype))
    n_params = len(in_names)
    n_outs = len(out_avals)
    all_in_names = in_names + out_names
    if partition_name is not None:
        all_in_names = all_in_names + [partition_name]

    def _body(*args):
        operands = list(args)
        if partition_name is not None:
            operands.append(bass2jax.partition_id_tensor())
        outs = bass2jax._bass_exec_p.bind(
            *operands,
            out_avals=tuple(out_avals),
            in_names=tuple(all_in_names),
            out_names=tuple(out_names),
            lowering_input_output_aliases=(),
            sim_require_finite=True,
            sim_require_nnan=True,
            nc=nc,
        )
        return tuple(outs)

    _body.__name__ = "_body"

    devices = jax.devices()[:n_cores]
    mesh = Mesh(np.asarray(devices), ("core",))
    spec = PartitionSpec("core")
    sharding = NamedSharding(mesh, spec)
    donate = tuple(range(n_params, n_params + n_outs))
    fn = jax.jit(
        shard_map(
            _body,
            mesh=mesh,
            in_specs=(spec,) * (n_params + n_outs),
            out_specs=(spec,) * n_outs,
            check_rep=False,
        ),
        donate_argnums=donate,
        keep_unused=True,
    )

    concat_in = [
        np.concatenate([np.asarray(in_maps[c][nm]) for c in range(n_cores)], axis=0)
        for nm in in_names
    ]
    concat_zero = [
        np.zeros((n_cores * z.shape[0], *z.shape[1:]), z.dtype) for z in zero_outs
    ]
    dev_in = [jax.device_put(a, sharding) for a in concat_in]
    for a in dev_in:
        a.block_until_ready()

    times = []
    for _ in range(iters + 1):
        dev_zero = [jax.device_put(z, sharding) for z in concat_zero]
        for z in dev_zero:
            z.block_until_ready()
        t0 = time.perf_counter()
        outs = fn(*dev_in, *dev_zero)
        for o in outs:
            o.block_until_ready()
        times.append(time.perf_counter() - t0)
    return times[1:]  # drop warmup/compile call


# revision 50
# speedup vs baseline: 52.0043x; 1.3816x over previous
"""AttLoRA MoE-routing kernel for 8 Trainium2 NeuronCores.

Reference computation (per problem nn_AttLoRAModule_85839216378078):
    base  = x @ W_org.T                                    [B,S,OUT]
    q     = x.mean(axis=1) @ Wq.T                          [B,K]
    coef  = softmax(q @ lora_keys.T / sqrt(K))             [B,E]
    h     = x @ lora_down[e]                               [B,S,E,R]
    delta = sum_e coef[b,e] * (h[...,e,:] @ lora_up[e])    [B,S,OUT]
    out   = base + delta * SCALE

Sharding: 8 cores = 4 batches x 2 OUT-halves.  Core c handles batch c//2,
output columns [(c%2)*2048, (c%2+1)*2048).  Each core sees the full x[b], so
the router (softmax coefficients) is computed on-device per core with no
collectives.

Device strategy (per core):
  - All matmuls in float32r (full PE rate at N>=512, ~1e-4 rel err).
  - K(=IN)-split into 2 passes of 2048 contraction rows so the x.T slab
    [2048, 2048] f32 (16 MiB) stays SBUF-resident per pass; W / lora_down
    stream through a shared pool; output accumulated in DRAM via a second
    pass with accum_op=add DMA.
  - LoRA path: tT[er,s] = (x @ lora_down).T accumulated across passes into a
    bf16 tile; router coeffs folded into lora_up tiles (bf16); 4 delta
    matmuls appended to each pass-2 PSUM accumulation group.
  - Router: proj = x @ ((Wq.T @ keys.T)/(S*sqrt(K))) accumulated in one PSUM
    bank across both passes, reduced over S, softmax on one partition, then
    broadcast to partitions via rank-1 outer-product matmuls.
"""

import math
import os

import numpy as np

import concourse.bacc as bacc
import concourse.mybir as mybir
import concourse.tile as tile
from concourse.bass_utils import run_bass_kernel_spmd

# Problem shapes (hardcoded per contest contract)
B, S, IN, OUT = 4, 2048, 4096, 4096
E, R, K = 8, 64, 128
ER = E * R            # 512
OH = OUT // 2         # 2048 output cols per core
P = 128
NP = 2                # contraction passes
IOP = IN // NP // P   # 16 io-subtiles per pass
SCALE = 1.0           # (alpha/lora_dim) * multiplier

F32 = mybir.dt.float32
F32R = mybir.dt.float32r
BF16 = mybir.dt.bfloat16

_NC_CACHE = {}


def _build_nc():
    nc = bacc.Bacc("TRN2", target_bir_lowering=False, debug=False)

    # f32r inputs: same 4-byte fp32 payload, but typed float32r end-to-end so
    # the BIR verifier accepts them as FP32r-matmul operands.
    xT = nc.dram_tensor("xT", [IN, S], F32R, kind="ExternalInput")
    wT = nc.dram_tensor("wT", [IN, OH], F32R, kind="ExternalInput")
    ldn = nc.dram_tensor("ldn", [IN, ER], F32R, kind="ExternalInput")
    lup = nc.dram_tensor("lup", [ER, OH], F32, kind="ExternalInput")
    mk = nc.dram_tensor("mk", [IN, E], F32R, kind="ExternalInput")
    cind = nc.dram_tensor("cind", [E, ER], F32, kind="ExternalInput")
    out = nc.dram_tensor("out", [S, OH], F32, kind="ExternalOutput")
    # pass-1 partial sums; read back and added during pass-2 eviction (avoids
    # SWDGE read-modify-write accumulate DMAs, which serialize on hardware)
    out_p1 = nc.dram_tensor("out_p1", [S, OH], F32)

    xT_ap, wT_ap, ldn_ap, lup_ap, mk_ap, cind_ap, out_ap, out_p1_ap = (
        t.ap() for t in (xT, wT, ldn, lup, mk, cind, out, out_p1)
    )

    trace_sim = os.environ.get("KERNEL_SIM_TRACE", "0") == "1"
    with tile.TileContext(nc, trace_sim=trace_sim) as tc:
        with (
            tc.tile_pool(name="xpool", bufs=1) as xpool,
            tc.tile_pool(name="spool", bufs=3) as spool,
            tc.tile_pool(name="tpool", bufs=1) as tpool,
            tc.tile_pool(name="lpool", bufs=2) as lpool,
            tc.tile_pool(name="opool", bufs=2) as opool,
            tc.tile_pool(name="rpool", bufs=1) as rpool,
            tc.tile_pool(name="ptp", bufs=2, space="PSUM") as ptp,
            tc.tile_pool(name="pop", bufs=4, space="PSUM") as pop,
            tc.tile_pool(name="prp", bufs=1, space="PSUM") as prp,
            tc.tile_pool(name="pccp", bufs=1, space="PSUM") as pccp,
        ):
            # --- persistent tiles ---
            mk_sb = rpool.tile([P, IN // P, E], F32R, name="mk_sb")
            nc.sync.dma_start(mk_sb[:], mk_ap.rearrange("(io pp) e -> pp io e", pp=P))
            cind_sb = rpool.tile([E, ER], F32, name="cind_sb")
            nc.sync.dma_start(cind_sb[:], cind_ap)

            tT = tpool.tile([P, ER // P, S], BF16, name="tT")
            pr_t = prp.tile([E, 512], F32, name="pr_t")
            ones8 = rpool.tile([E, 1], F32, name="ones8")
            nc.any.memset(ones8[:], 1.0)
            ones_row = rpool.tile([1, P], F32, name="ones_row")
            nc.any.memset(ones_row[:], 1.0)
            coeff_cols = rpool.tile([P, ER // P], F32, name="coeff_cols")

            SC = S // 512  # 4 s-chunks

            for p in range(NP):
                i0 = p * (IN // NP)
                # --- phase-T weights first: the very first matmul needs ldc0 ---
                def load_ldc(j):
                    ldcj = spool.tile([P, 8, ER], F32R, tag="stream", name=f"ldc_{p}_{j}")
                    nc.sync.dma_start(
                        ldcj[:],
                        ldn_ap[i0 + j * 1024 : i0 + (j + 1) * 1024, :].rearrange(
                            "(io pp) e -> pp io e", pp=P
                        ),
                    )
                    return ldcj

                ldcs = [load_ldc(0)]

                # --- load x in eighths (each [256 i, S]) for DMA-queue
                # parallelism and early phase-T start ---
                xq = []
                for k in range(8):
                    xqk = xpool.tile([P, 2, S], F32R, tag=f"xq{k}", name=f"xq{k}_{p}")
                    # alternate SWDGE/HWDGE so x streams over both DMA paths;
                    # xq0 goes on gpsimd so it loads in parallel with ldc0 (sync)
                    eng = nc.gpsimd if k % 2 == 0 else nc.sync
                    eng.dma_start(
                        xqk[:],
                        xT_ap[i0 + k * 256 : i0 + (k + 1) * 256, :].rearrange(
                            "(io pp) s -> pp io s", pp=P
                        ),
                    )
                    xq.append(xqk)

                def xs(io, fslice):
                    return xq[io // 2][:, io % 2, fslice]

                ldcs.append(load_ldc(1))  # needed only at phase-T midpoint

                # --- phase T: tT += (ldn_pass.T @ x_pass), chunked by 8 io ---
                for j in range(2):
                    ldc = ldcs[j]
                    for u in range(ER // P):
                        for c in range(SC):
                            ps = ptp.tile([P, 512], F32, tag="pt", name=f"pt_{p}_{j}_{u}_{c}")
                            for jo in range(8):
                                io = j * 8 + jo
                                nc.tensor.matmul(
                                    ps[:],
                                    ldc[:, jo, u * P : (u + 1) * P],
                                    xs(io, slice(c * 512, (c + 1) * 512)),
                                    start=(jo == 0),
                                    stop=(jo == 7),
                                )
                            dst = tT[:, u, c * 512 : (c + 1) * 512]
                            if p == 0 and j == 0:
                                nc.vector.tensor_copy(dst, ps[:])
                            else:
                                nc.vector.tensor_tensor(
                                    dst, dst, ps[:], mybir.AluOpType.add
                                )

                # --- router projection: pr_t[e, j] += sum_i x[s,i] mk[i,e] ---
                for c in range(SC):
                    for io in range(IOP):
                        nc.tensor.matmul(
                            pr_t[:],
                            mk_sb[:, p * IOP + io, :],
                            xs(io, slice(c * 512, (c + 1) * 512)),
                            start=(p == 0 and c == 0 and io == 0),
                            stop=(p == NP - 1 and c == SC - 1 and io == IOP - 1),
                        )

                if p == NP - 1:
                    # --- router finalize (on-device softmax) ---
                    scores = rpool.tile([E, 1], F32, name="scores")
                    nc.vector.reduce_sum(scores[:], pr_t[:], axis=mybir.AxisListType.X)
                    exps = rpool.tile([E, 1], F32, name="exps")
                    nc.scalar.activation(
                        exps[:], scores[:], mybir.ActivationFunctionType.Exp
                    )
                    # sum(exp) via PE partition reduction -> [1, 1]
                    psum_s = pccp.tile([1, 1], F32, tag="pcc", name="psum_s")
                    nc.tensor.matmul(psum_s[:], exps[:], ones8[:], start=True, stop=True)
                    rinv = rpool.tile([1, 1], F32, name="rinv")
                    nc.vector.reciprocal(rinv[:], psum_s[:])
                    # broadcast 1/sum to all 128 partitions via rank-1 outer product
                    rb_p = pccp.tile([P, 1], F32, tag="pcc", name="rb_p")
                    nc.tensor.matmul(rb_p[:], ones_row[:], rinv[:], start=True, stop=True)
                    rb = rpool.tile([P, 1], F32, name="rb")
                    nc.vector.tensor_copy(rb[:], rb_p[:])
                    # partition placement: cc_un[pp, u] = exp(score[(u*128+pp)//64])
                    cc_un = rpool.tile([P, ER // P], F32, name="cc_un")
                    for u in range(ER // P):
                        pcc = pccp.tile([P, 1], F32, tag="pcc", name=f"pcc_{u}")
                        nc.tensor.matmul(
                            pcc[:],
                            cind_sb[:, u * P : (u + 1) * P],
                            exps[:],
                            start=True,
                            stop=True,
                        )
                        nc.vector.tensor_copy(cc_un[:, u : u + 1], pcc[:])
                    # coeff_cols = cc_un / sum(exp) * SCALE  (SCALE == 1.0)
                    nc.vector.tensor_scalar_mul(coeff_cols[:], cc_un[:], rb[:])

                lsc_tiles = [None] * (OH // 512)

                def load_lsc(nn_):
                    lraw = spool.tile(
                        [P, ER // P, 512], F32, tag="stream", name=f"lraw_{nn_}"
                    )
                    nc.gpsimd.dma_start(
                        lraw[:],
                        lup_ap[:, nn_ * 512 : (nn_ + 1) * 512].rearrange(
                            "(u pp) o -> pp u o", pp=P
                        ),
                    )
                    t = lpool.tile([P, ER // P, 512], BF16, tag="lsc", name=f"lsc_{nn_}")
                    nc.vector.tensor_tensor(
                        t[:],
                        lraw[:],
                        coeff_cols[:, :, None].to_broadcast((P, ER // P, 512)),
                        mybir.AluOpType.mult,
                    )
                    lsc_tiles[nn_] = t

                if p == NP - 1:
                    load_lsc(0)

                # --- main output loop ---
                for n in range(OH // 512):
                    wc = []
                    for j in range(2):
                        wcj = spool.tile(
                            [P, 8, 512], F32R, tag="stream", name=f"wc_{p}_{n}_{j}"
                        )
                        (nc.sync if j == 0 else nc.gpsimd).dma_start(
                            wcj[:],
                            wT_ap[
                                i0 + j * 1024 : i0 + (j + 1) * 1024,
                                n * 512 : (n + 1) * 512,
                            ].rearrange("(io pp) o -> pp io o", pp=P),
                        )
                        wc.append(wcj)
                    if p == NP - 1:
                        lsc = lsc_tiles[n]
                        if n + 1 < OH // 512:
                            # prefetch next n's scaled lup during this m-loop
                            load_lsc(n + 1)

                    for m in range(S // P):
                        sl = (
                            slice(m * P, (m + 1) * P),
                            slice(n * 512, (n + 1) * 512),
                        )
                        if p == NP - 1:
                            oprev = opool.tile(
                                [P, 512], F32, tag="oprev", name=f"opr_{n}_{m}", bufs=1
                            )
                            nc.sync.dma_start(oprev[:], out_p1_ap[sl])
                        po_t = pop.tile([P, 512], F32, tag="po", name=f"po_{p}_{n}_{m}")
                        for io in range(IOP):
                            nc.tensor.matmul(
                                po_t[:],
                                xs(io, slice(m * P, (m + 1) * P)),
                                wc[io // 8][:, io % 8, :],
                                start=(io == 0),
                                stop=(io == IOP - 1 and p < NP - 1),
                            )
                        if p == NP - 1:
                            for u in range(ER // P):
                                nc.tensor.matmul(
                                    po_t[:],
                                    tT[:, u, m * P : (m + 1) * P],
                                    lsc[:, u, :],
                                    start=False,
                                    stop=(u == ER // P - 1),
                                )
                        ost = opool.tile(
                            [P, 512], F32, tag="ost", name=f"ost_{p}_{n}_{m}", bufs=1
                        )
                        if p == 0:
                            nc.vector.tensor_copy(ost[:], po_t[:])
                            (nc.sync if m % 2 == 0 else nc.gpsimd).dma_start(
                                out_p1_ap[sl], ost[:]
                            )
                        else:
                            nc.vector.tensor_tensor(
                                ost[:], oprev[:], po_t[:], mybir.AluOpType.add
                            )
                            (nc.sync if m % 2 == 0 else nc.gpsimd).dma_start(
                                out_ap[sl], ost[:]
                            )

    nc.compile()
    return nc


def kernel(x, W_org, lora_down, lora_up, lora_keys, Wq):
    x = np.ascontiguousarray(np.asarray(x, dtype=np.float32))
    W_org = np.asarray(W_org, dtype=np.float32)
    lora_down = np.asarray(lora_down, dtype=np.float32)
    lora_up = np.asarray(lora_up, dtype=np.float32)
    lora_keys = np.asarray(lora_keys, dtype=np.float32)
    Wq = np.asarray(Wq, dtype=np.float32)

    # Host-side constant folding / layout prep (transposes to K-major)
    wT_full = np.ascontiguousarray(W_org.T)                          # [IN, OUT]
    ldn = np.ascontiguousarray(lora_down.transpose(1, 0, 2).reshape(IN, ER))
    lup_full = np.ascontiguousarray(lora_up.reshape(ER, OUT))
    mk = np.ascontiguousarray(
        (Wq.T @ lora_keys.T) / (S * math.sqrt(K))
    ).astype(np.float32)                                             # [IN, E]
    cind = np.repeat(np.eye(E, dtype=np.float32), R, axis=1)         # [E, ER]
    xT = [np.ascontiguousarray(x[b].T) for b in range(B)]            # [IN, S]
    wT_half = [np.ascontiguousarray(wT_full[:, h * OH : (h + 1) * OH]) for h in range(2)]
    lup_half = [np.ascontiguousarray(lup_full[:, h * OH : (h + 1) * OH]) for h in range(2)]

    if "nc" not in _NC_CACHE:
        _NC_CACHE["nc"] = _build_nc()
    nc = _NC_CACHE["nc"]

    in_maps = []
    for c in range(8):
        b, h = c // 2, c % 2
        in_maps.append(
            {
                "xT": xT[b],
                "wT": wT_half[h],
                "ldn": ldn,
                "lup": lup_half[h],
                "mk": mk,
                "cind": cind,
            }
        )

    res = run_bass_kernel_spmd(nc, in_maps, core_ids=list(range(8)), trace=False)
    _NC_CACHE["last_result"] = res
    _NC_CACHE["last_in_maps"] = in_maps

    outp = np.empty((B, S, OUT), dtype=np.float32)
    for c in range(8):
        b, h = c // 2, c % 2
        outp[b, :, h * OH : (h + 1) * OH] = res.results[c]["out"]
    return outp


def _build_baseline_nc():
    """Same I/O signature as the real kernel, near-zero device work.

    Used to measure the fixed dispatch/relay overhead of one execution so the
    real kernel's device time can be estimated as (full - baseline)."""
    nc = bacc.Bacc("TRN2", target_bir_lowering=False, debug=False)
    xT = nc.dram_tensor("xT", [IN, S], F32R, kind="ExternalInput")
    wT = nc.dram_tensor("wT", [IN, OH], F32R, kind="ExternalInput")
    ldn = nc.dram_tensor("ldn", [IN, ER], F32R, kind="ExternalInput")
    lup = nc.dram_tensor("lup", [ER, OH], F32, kind="ExternalInput")
    mk = nc.dram_tensor("mk", [IN, E], F32R, kind="ExternalInput")
    cind = nc.dram_tensor("cind", [E, ER], F32, kind="ExternalInput")
    out = nc.dram_tensor("out", [S, OH], F32, kind="ExternalOutput")
    with tile.TileContext(nc) as tc:
        with tc.tile_pool(name="bp", bufs=1) as bp:
            t = bp.tile([P, 512], F32, name="t")
            nc.sync.dma_start(t[:], lup.ap()[:P, :512])
            nc.sync.dma_start(out.ap()[:P, :512], t[:])
            _ = (xT, wT, ldn, mk, cind)
    nc.compile()
    return nc


def benchmark_baseline(iters: int = 8):
    if "bnc" not in _NC_CACHE:
        _NC_CACHE["bnc"] = _build_baseline_nc()
    return benchmark(iters, nc=_NC_CACHE["bnc"])


def benchmark(iters: int = 8, nc=None):
    """Time device execution with inputs pre-placed on the 8 cores.

    Mirrors bass2jax.run_bass_via_pjrt's multi-core shard_map path but keeps
    the non-donated inputs resident on device so the timed region is
    dispatch + NEFF execution only.  Returns per-iteration seconds.
    """
    import time

    import jax
    from jax.experimental.shard_map import shard_map
    from jax.sharding import Mesh, NamedSharding, PartitionSpec

    from concourse import bass2jax, mybir as _mybir

    if nc is None:
        nc = _NC_CACHE["nc"]
    in_maps = _NC_CACHE["last_in_maps"]
    n_cores = len(in_maps)

    bass2jax.install_neuronx_cc_hook()

    partition_name = nc.partition_id_tensor.name if nc.partition_id_tensor else None
    in_names, out_names, out_avals, zero_outs = [], [], [], []
    for alloc in nc.m.functions[0].allocations:
        if not isinstance(alloc, _mybir.MemoryLocationSet):
            continue
        name = alloc.memorylocations[0].name
        if alloc.kind == "ExternalInput":
            if name != partition_name:
                in_names.append(name)
        elif alloc.kind == "ExternalOutput":
            aval = jax.core.ShapedArray(
                tuple(alloc.tensor_shape), _mybir.dt.np(alloc.dtype)
            )
            out_avals.append(aval)
            out_names.append(name)
            zero_outs.append(np.zeros(aval.shape, aval.dtype))
    n_params = len(in_names)
    n_outs = len(out_avals)
    all_in_names = in_names + out_names
    if partition_name is not None:
        all_in_names = all_in_names + [partition_name]

    def _body(*args):
        operands = list(args)
        if partition_name is not None:
            operands.append(bass2jax.partition_id_tensor())
        outs = bass2jax._bass_exec_p.bind(
            *operands,
            out_avals=tuple(out_avals),
            in_names=tuple(all_in_names),
            out_names=tuple(out_names),
            lowering_input_output_aliases=(),
            sim_require_finite=True,
            sim_require_nnan=True,
            nc=nc,
        )
        return tuple(outs)

    _body.__name__ = "_body"

    devices = jax.devices()[:n_cores]
    mesh = Mesh(np.asarray(devices), ("core",))
    spec = PartitionSpec("core")
    sharding = NamedSharding(mesh, spec)
    donate = tuple(range(n_params, n_params + n_outs))
    fn = jax.jit(
        shard_map(
            _body,
            mesh=mesh,
            in_specs=(spec,) * (n_params + n_outs),
            out_specs=(spec,) * n_outs,
            check_rep=False,
        ),
        donate_argnums=donate,
        keep_unused=True,
    )

    concat_in = [
        np.concatenate([np.asarray(in_maps[c][nm]) for c in range(n_cores)], axis=0)
        for nm in in_names
    ]
    concat_zero = [
        np.zeros((n_cores * z.shape[0], *z.shape[1:]), z.dtype) for z in zero_outs
    ]
    dev_in = [jax.device_put(a, sharding) for a in concat_in]
    for a in dev_in:
        a.block_until_ready()

    times = []
    for _ in range(iters + 1):
        dev_zero = [jax.device_put(z, sharding) for z in concat_zero]
        for z in dev_zero:
            z.block_until_ready()
        t0 = time.perf_counter()
        outs = fn(*dev_in, *dev_zero)
        for o in outs:
            o.block_until_ready()
        times.append(time.perf_counter() - t0)
    return times[1:]  # drop warmup/compile call
